# revision 3
# baseline (speedup 1.0000x reference)
"""Trainium2 Bass kernel for nn_AdaptiveGraphConvolutionalLSTM.

Reference computation (B=32, F=1024):
    gc_R  = concat_k( einsum('bf,bfg->bg', x, GC_Rk_w  * Rk_i) )   k=1..3
    gc_NR = concat_k( einsum('bf,bfg->bg', x, GC_NRk_w * Rk_i) )   (NR reuses R masks)
    combined = concat([gc_R, gc_NR, hidden])                        # [B, 7F]
    f,i,o = sigmoid(combined @ {fl,il,ol}_w.T + b); C = tanh(combined @ Cl_w.T + Cl_b)
    Cell = f*cell + i*C ; Hidden = o*tanh(Cell)

Distribution over 8 NeuronCores (better than plain batch-DP):
  - the GC output dim g and the gate output columns are sharded across
    cores (128 columns each); the NR*_i masks are never read (unused by
    the reference), so per-core HBM traffic is ~33 MiB instead of the
    ~184 MiB a replicated-weight data-parallel split would need.
  - per (mask k, fc=f/128 chunk): VectorE computes u = mask * w in bf16
    (2x DVE mode) with the weight broadcast over the batch dim; PE then
    contracts over f with x^T as the stationary operand (M=16 half-batch),
    which computes x*mask*w reduced over f for every (b, b', g) pair; the
    useful diagonal (b==b') is extracted through a small DRAM bounce.
  - a 48 KB/rank AllGather shares the per-core combined^T slices, a
    single xbar-transpose DMA turns the gathered activations into
    [g, b] layout, and the gate matmuls run with column-sharded weights
    (stationary = combined^T chunk, moving = gate weight chunk) so gates,
    LSTM elementwise, and outputs stay in natural [b, col] layout.

kernel(**inputs) takes the FULL inputs and returns (Hidden, Cell) full.
"""
import numpy as np
import ml_dtypes

from concourse import bass, bacc, tile, mybir
from concourse.bass_utils import run_bass_kernel_spmd

BF16 = ml_dtypes.bfloat16
B, F, K, NCORES = 32, 1024, 3, 8
P = 128          # partitions / f-chunk size
FC = F // P      # 8 f-chunks
G = F // NCORES  # 128 g-columns per core
NKC = 7 * FC + 1  # 57 gate contraction chunks (48 gathered + 8 hidden + bias)
HB = B // 2      # 16, half batch (GC psum M)

_DT_BF = mybir.dt.bfloat16
_DT_F32 = mybir.dt.float32


def build_nc(reps: int = 1):
    """Build the SPMD per-core program. reps>1 repeats the whole compute
    body (for timing); reps=1 is the real kernel."""
    nc = bacc.Bacc("TRN2", target_bir_lowering=False, debug=False,
                   num_devices=NCORES)

    p_m = nc.dram_tensor("m", [K, FC, P, B * G], _DT_BF, kind="ExternalInput")
    p_w6 = nc.dram_tensor("w6", [P, 6 * FC * G], _DT_BF, kind="ExternalInput")
    p_xT = nc.dram_tensor("xT", [P, FC * B], _DT_BF, kind="ExternalInput")
    p_hp = nc.dram_tensor("hp", [P, 9 * B], _DT_BF, kind="ExternalInput")
    p_gw = nc.dram_tensor("gw", [4, P, NKC * P], _DT_BF, kind="ExternalInput")
    p_cs = nc.dram_tensor("cs", [B, G], _DT_F32, kind="ExternalInput")
    p_out = nc.dram_tensor("out", [2, B, G], _DT_F32, kind="ExternalOutput")

    AF = mybir.ActivationFunctionType
    with tile.TileContext(nc) as tc:
        with tc.tile_pool(name="mp", bufs=10) as mp, \
             tc.tile_pool(name="up", bufs=3) as up, \
             tc.tile_pool(name="cst", bufs=1) as cst, \
             tc.tile_pool(name="stg", bufs=2) as stg, \
             tc.tile_pool(name="sml", bufs=2) as sml, \
             tc.tile_pool(name="pgc", bufs=1, space="PSUM") as pgc, \
             tc.tile_pool(name="pgt", bufs=1, space="PSUM") as pgt, \
             tc.tile_pool(name="dsc", bufs=2, space="DRAM") as dsc, \
             tc.tile_pool(name="dcc", bufs=2, space="DRAM") as dcc:

            # persistent loads
            w6_t = cst.tile([P, 6 * FC * G], _DT_BF, tag="w6")
            nc.sync.dma_start(out=w6_t[:, :], in_=p_w6[:, :])
            xT_t = cst.tile([P, FC * B], _DT_BF, tag="xT")
            nc.sync.dma_start(out=xT_t[:, :], in_=p_xT[:, :])
            hp_t = cst.tile([P, 9 * B], _DT_BF, tag="hp")
            nc.sync.dma_start(out=hp_t[:, :], in_=p_hp[:, :])
            cs_t = cst.tile([B, G], _DT_F32, tag="cs")
            nc.sync.dma_start(out=cs_t[:, :], in_=p_cs[:, :])
            # gate weights, 2 tiles per gate: A = chunks 0..28, B = 29..56
            gwA, gwB = [], []
            for g_i in range(4):
                ta = cst.tile([P, 29 * P], _DT_BF, tag=f"gwA{g_i}")
                nc.sync.dma_start(out=ta[:, :], in_=p_gw[g_i, :, :29 * P])
                gwA.append(ta)
                tb = cst.tile([P, 28 * P], _DT_BF, tag=f"gwB{g_i}")
                nc.sync.dma_start(out=tb[:, :], in_=p_gw[g_i, :, 29 * P:])
                gwB.append(tb)

            def gw_slice(g_i, kc):
                if kc < 29:
                    return gwA[g_i][:, kc * P:(kc + 1) * P]
                return gwB[g_i][:, (kc - 29) * P:(kc - 28) * P]

            for rep in range(reps):
                # gate psum accumulators [B, G] f32, one bank each
                pg = [pgt.tile([B, G], _DT_F32, tag=f"pg{g_i}",
                               name=f"pg{g_i}")
                      for g_i in range(4)]

                # ---- phase A: hidden-state + bias gate partials ----
                for kc in range(48, 57):
                    lhs = hp_t[:, (kc - 48) * B:(kc - 47) * B]
                    for g_i in range(4):
                        nc.tensor.matmul(pg[g_i][:, :], lhsT=lhs,
                                         rhs=gw_slice(g_i, kc),
                                         start=(kc == 48), stop=False)

                # ---- phase B: graph convolutions ----
                contrib = dcc.tile([6 * B, G], _DT_BF, tag="contrib")
                for k in range(K):
                    m_tiles = []
                    for fc in range(FC):
                        mt = mp.tile([P, B * G], _DT_BF, tag="m")
                        nc.sync.dma_start(out=mt[:, :], in_=p_m[k, fc, :, :])
                        m_tiles.append(mt)
                    for br in range(2):          # 0 = R branch, 1 = NR branch
                        w_i = br * 3 + k
                        for h in range(2):       # batch half
                            psum = pgc.tile([HB, HB * G], _DT_F32, tag="gc")
                            for fc in range(FC):
                                u = up.tile([P, HB * G], _DT_BF, tag="u")
                                w_ap = (w6_t[:, (w_i * FC + fc) * G:
                                             (w_i * FC + fc + 1) * G]
                                        .unsqueeze(1).broadcast_to([P, HB, G]))
                                m_ap = (m_tiles[fc][:, h * HB * G:(h + 1) * HB * G]
                                        .rearrange("p (b g) -> p b g", g=G))
                                u_ap = u[:, :].rearrange("p (b g) -> p b g", g=G)
                                nc.vector.tensor_mul(u_ap, m_ap, w_ap)
                                lhs = xT_t[:, fc * B + h * HB:
                                           fc * B + (h + 1) * HB]
                                for s in range(4):
                                    nc.tensor.matmul(
                                        psum[:, s * 512:(s + 1) * 512],
                                        lhsT=lhs, rhs=u[:, s * 512:(s + 1) * 512],
                                        start=(fc == 0), stop=(fc == FC - 1))
                            # extract diagonal b==b' via DRAM bounce
                            stage = stg.tile([HB, HB * G], _DT_BF, tag="stage")
                            nc.scalar.activation(stage[:, :], psum[:, :], AF.Copy)
                            scr = dsc.tile([HB, HB * G], _DT_BF, tag="scr")
                            nc.sync.dma_start(out=scr[:, :], in_=stage[:, :])
                            scr_ap = scr[:, :]
                            diag = bass.AP(scr_ap.tensor, scr_ap.offset,
                                           [[HB * G + G, HB], [1, G]])
                            j = br * 3 + k
                            nc.sync.dma_start(
                                out=contrib[j * B + h * HB:j * B + (h + 1) * HB, :],
                                in_=diag)

                # ---- phase C: AllGather the combined^T contributions ----
                gathered = dcc.tile([NCORES * 6 * B, G], _DT_BF, tag="gath",
                                    addr_space="Shared")
                nc.gpsimd.collective_compute(
                    "AllGather", mybir.AluOpType.bypass,
                    replica_groups=[list(range(NCORES))],
                    ins=[contrib.opt()], outs=[gathered.opt()])

                # ---- phase D: transpose to [g, (core,chunk,b)] ----
                combT = stg.tile([P, NCORES * 6 * B], _DT_BF, tag="combT")
                nc.sync.dma_start_transpose(combT[:, :], gathered[:, :])

                # ---- phase E: gathered gate matmuls ----
                for kc in range(48):
                    lhs = combT[:, kc * B:(kc + 1) * B]
                    for g_i in range(4):
                        nc.tensor.matmul(pg[g_i][:, :], lhsT=lhs,
                                         rhs=gw_slice(g_i, kc),
                                         start=False, stop=(kc == 47))

                # ---- phase F: LSTM cell ----
                f_t = sml.tile([B, G], _DT_F32, tag="f")
                i_t = sml.tile([B, G], _DT_F32, tag="i")
                o_t = sml.tile([B, G], _DT_F32, tag="o")
                C_t = sml.tile([B, G], _DT_F32, tag="C")
                nc.scalar.activation(f_t[:, :], pg[0][:, :], AF.Sigmoid)
                nc.scalar.activation(i_t[:, :], pg[1][:, :], AF.Sigmoid)
                nc.scalar.activation(o_t[:, :], pg[2][:, :], AF.Sigmoid)
                nc.scalar.activation(C_t[:, :], pg[3][:, :], AF.Tanh)
                t1 = sml.tile([B, G], _DT_F32, tag="t1")
                nc.vector.tensor_mul(t1[:, :], f_t[:, :], cs_t[:, :])
                t2 = sml.tile([B, G], _DT_F32, tag="t2")
                nc.vector.tensor_mul(t2[:, :], i_t[:, :], C_t[:, :])
                cell = sml.tile([B, G], _DT_F32, tag="cell")
                nc.vector.tensor_add(cell[:, :], t1[:, :], t2[:, :])
                tc_t = sml.tile([B, G], _DT_F32, tag="tc")
                nc.scalar.activation(tc_t[:, :], cell[:, :], AF.Tanh)
                hid = sml.tile([B, G], _DT_F32, tag="hid")
                nc.vector.tensor_mul(hid[:, :], o_t[:, :], tc_t[:, :])
                nc.sync.dma_start(out=p_out[0, :, :], in_=hid[:, :])
                nc.sync.dma_start(out=p_out[1, :, :], in_=cell[:, :])

    nc.compile()
    return nc


def _bf(a):
    return np.ascontiguousarray(a.astype(BF16))


def prep_in_maps(input, R1_i, R2_i, R3_i, Hidden_State, Cell_State,
                 GC_R1_w, GC_R2_w, GC_R3_w, GC_NR1_w, GC_NR2_w, GC_NR3_w,
                 fl_w, fl_b, il_w, il_b, ol_w, ol_b, Cl_w, Cl_b):
    """Shard + relayout all inputs for the 8 cores (host side)."""
    input = np.asarray(input, np.float32)
    masks = [np.asarray(m, np.float32) for m in (R1_i, R2_i, R3_i)]
    hs = np.asarray(Hidden_State, np.float32)
    cs = np.asarray(Cell_State, np.float32)
    gcw = [np.asarray(w, np.float32) for w in
           (GC_R1_w, GC_R2_w, GC_R3_w, GC_NR1_w, GC_NR2_w, GC_NR3_w)]
    gates = [(np.asarray(w, np.float32), np.asarray(b, np.float32))
             for w, b in ((fl_w, fl_b), (il_w, il_b), (ol_w, ol_b),
                          (Cl_w, Cl_b))]

    # replicated tensors
    xT = _bf(input.T.reshape(FC, P, B).transpose(1, 0, 2).reshape(P, FC * B))
    hT = hs.T.reshape(FC, P, B).transpose(1, 0, 2).reshape(P, FC * B)
    bias_blk = np.zeros((P, B), np.float32)
    bias_blk[0, :] = 1.0
    hp = _bf(np.concatenate([hT, bias_blk], axis=1))

    # gathered-feature permutation (for gate weight rows): (core, chunk, g)
    feat = np.empty(48 * P, np.int64)
    idx = 0
    for c2 in range(NCORES):
        for j in range(6):
            base = (j * F if j < 3 else 3 * F + (j - 3) * F) + c2 * G
            feat[idx:idx + G] = np.arange(base, base + G)
            idx += G
    h_feat = np.arange(6 * F, 7 * F)

    in_maps = []
    for c in range(NCORES):
        gsl = slice(c * G, (c + 1) * G)
        m = np.empty((K, FC, P, B * G), BF16)
        for k in range(K):
            t = masks[k][:, :, gsl].transpose(1, 0, 2)      # [F, B, G]
            m[k] = _bf(t.reshape(FC, P, B * G))
        w6 = np.empty((P, 6 * FC * G), BF16)
        for w_i, W in enumerate(gcw):
            blk = W[:, gsl].reshape(FC, P, G).transpose(1, 0, 2)
            w6[:, w_i * FC * G:(w_i + 1) * FC * G] = _bf(blk.reshape(P, FC * G))
        gw = np.empty((4, P, NKC * P), BF16)
        for g_i, (W, bv) in enumerate(gates):
            Wc = W[gsl, :]                                   # [G(out), 7F]
            gpart = Wc[:, feat].T.reshape(48, P, P)          # [kc, kk, m]
            hpart = Wc[:, h_feat].T.reshape(FC, P, P)
            bias_chunk = np.zeros((1, P, P), np.float32)
            bias_chunk[0, 0, :] = bv[gsl]
            allc = np.concatenate([gpart, hpart, bias_chunk], axis=0)
            gw[g_i] = _bf(allc.transpose(1, 0, 2).reshape(P, NKC * P))
        in_maps.append({
            "m": m, "w6": w6, "xT": xT, "hp": hp, "gw": gw,
            "cs": np.ascontiguousarray(cs[:, gsl]),
        })
    return in_maps


_cached_nc = None


def kernel(**inputs):
    """Full inputs in, full outputs out. Shards across 8 NeuronCores."""
    global _cached_nc
    inputs = {k: np.asarray(v) for k, v in inputs.items()}
    # NR1_i/NR2_i/NR3_i are accepted but unused (reference reuses R masks)
    args = {k: inputs[k] for k in (
        "input", "R1_i", "R2_i", "R3_i", "Hidden_State", "Cell_State",
        "GC_R1_w", "GC_R2_w", "GC_R3_w", "GC_NR1_w", "GC_NR2_w", "GC_NR3_w",
        "fl_w", "fl_b", "il_w", "il_b", "ol_w", "ol_b", "Cl_w", "Cl_b")}
    in_maps = prep_in_maps(**args)
    if _cached_nc is None:
        _cached_nc = build_nc(reps=1)
    res = run_bass_kernel_spmd(_cached_nc, in_maps,
                               core_ids=list(range(NCORES)))
    hidden = np.empty((B, F), np.float32)
    cell = np.empty((B, F), np.float32)
    for c in range(NCORES):
        o = res.results[c]["out"]
        hidden[:, c * G:(c + 1) * G] = o[0]
        cell[:, c * G:(c + 1) * G] = o[1]
    return hidden, cell
